# revision 40
# baseline (speedup 1.0000x reference)
"""Trainium2 Bass kernel for nn_Attention_17334488007364.

Computation (per batch element, x as [C=128, N=4096]):
    q = wq @ x                      [16, 4096]
    k = maxpool2(wk @ x)            [16, 1024]
    v = maxpool2(wv @ x)            [64, 1024]
    attn = softmax(q^T k, axis=m)   [4096, 1024]
    o = v @ attn^T                  [64, 4096]
    out = gamma * (wo @ o) + x      [128, 4096]

Sharding: pure data parallel — B=16 over 8 cores, 2 batch elements/core.

Per-core dataflow ("transposed" layout, m on partitions everywhere):
  - qkv projection fused into one bf16 matmul; x is cast f32->bf16 by a
    gpsimd DMA straight from DRAM (gpsimd DMAs can cast)
  - scores^T[m_tile][128, n] = k_tile^T q  (bf16, K=16; 4-way row-group
    packing via tile_position so 4 n-chunks compute concurrently)
  - p = exp(scores) on ACT straight out of PSUM in [128, 2048] calls
    (ACT is the bottleneck: 8.4M exps/core ~= 55us floor)
  - o_u[65, n-chunk] accumulates sum_m vT~[128,65].T @ p where vT~ has a
    trailing ones column, so row 64 = softmax denominator for free
  - denominators of 4 chunks are packed to partitions 0..3 via DMA and
    inverted by ONE DVE reciprocal per half (native recip cost is mostly
    per-call, so packing is ~4x cheaper)
  - 1/den broadcast to 64 partitions via a DRAM round-trip DMA; then
    onorm = (ou * gamma) * rden in one scalar_tensor_tensor, wo matmul,
    and residual out = o2 + x in one tensor_tensor.
"""

from contextlib import ExitStack

import numpy as np

import concourse.bacc as bacc
import concourse.mybir as mybir
from concourse import masks
from concourse.alu_op_type import AluOpType
from concourse.tile import TileContext

FP32 = mybir.dt.float32
BF16 = mybir.dt.bfloat16
AFT = mybir.ActivationFunctionType

# Per-core problem shape (hardcoded; harness provides full inputs).
B_FULL, C, H, W = 16, 128, 64, 64
N_CORES = 8
B_LOC = B_FULL // N_CORES            # 2
HW = H * W                           # 4096
M = HW // 4                          # 1024 (after 2x2 maxpool)
CQ, CV = C // 8, C // 2              # 16, 64
NCH = 512                            # psum-bank-sized n chunk
NCHUNKS = HW // NCH                  # 8
EXPSPAN = 2048                       # 4 banks per exp call
MT = M // 128                        # 8 m-tiles of 128

PACK = True                          # 4-way row-group packing for scores


def build_nc():
    nc = bacc.Bacc()
    x_e = nc.declare_dram_parameter("x", [B_LOC, C, HW], FP32, isOutput=False)
    wq_e = nc.declare_dram_parameter("wq", [CQ, C], FP32, isOutput=False)
    wk_e = nc.declare_dram_parameter("wk", [CQ, C], FP32, isOutput=False)
    wv_e = nc.declare_dram_parameter("wv", [CV, C], FP32, isOutput=False)
    wo_e = nc.declare_dram_parameter("wo", [C, CV], FP32, isOutput=False)
    g_e = nc.declare_dram_parameter("gamma", [1], FP32, isOutput=False)
    out_e = nc.declare_dram_parameter("out", [B_LOC, C, HW], FP32, isOutput=True)

    with TileContext(nc) as tc, ExitStack() as ctx:
        const = ctx.enter_context(tc.tile_pool(name="const", bufs=1))
        xpool = ctx.enter_context(tc.tile_pool(name="x", bufs=2))
        qkv = ctx.enter_context(tc.tile_pool(name="qkv", bufs=2))
        ppool = ctx.enter_context(tc.tile_pool(name="p", bufs=17))
        vtpool = ctx.enter_context(tc.tile_pool(name="vt", bufs=18))
        eppool = ctx.enter_context(tc.tile_pool(name="ep", bufs=3))
        outpool = ctx.enter_context(tc.tile_pool(name="outp", bufs=3))
        # PSUM budget (8 banks): scores 4 + av 2 + w 1 + m 1
        ps_s = ctx.enter_context(tc.tile_pool(name="ps_s", bufs=3, space="PSUM"))
        ps_av = ctx.enter_context(tc.tile_pool(name="ps_av", bufs=1, space="PSUM"))
        ps_w = ctx.enter_context(tc.tile_pool(name="ps_w", bufs=1, space="PSUM"))
        ps_m = ps_w
        dscratch = ctx.enter_context(tc.tile_pool(name="dscr", bufs=4, space="DRAM"))

        # ---------------- constants / weight preprocessing ----------------
        ident = const.tile([128, 128], FP32)
        masks.make_identity(nc, ident[:])
        ident_bf = const.tile([128, 128], BF16)
        masks.make_identity(nc, ident_bf[:])

        wq_sb = const.tile([CQ, C], FP32, tag="wq")
        wk_sb = const.tile([CQ, C], FP32, tag="wk")
        wv_sb = const.tile([CV, C], FP32, tag="wv")
        wo_sb = const.tile([C, CV], FP32, tag="wo")
        nc.sync.dma_start(wq_sb[:], wq_e[:])
        nc.sync.dma_start(wk_sb[:], wk_e[:])
        nc.sync.dma_start(wv_sb[:], wv_e[:])
        nc.sync.dma_start(wo_sb[:], wo_e[:])

        # gamma broadcast to all 128 partitions: [128, 1]
        g_sb = const.tile([128, 1], FP32, tag="g")
        nc.gpsimd.dma_start(
            g_sb[:, 0:1], g_e[:].unsqueeze(0).partition_broadcast(128)
        )

        # W_cat^T: cols 0:16 = wq^T, 32:48 = wk^T, 64:128 = wv^T (32-aligned
        # so PSUM consumer slices start at partition 0/32/64)
        ps_wt = ps_w.tile([128, NCH], FP32, tag="wm")
        nc.tensor.transpose(ps_wt[:, 0:CQ], wq_sb[:], ident[0:CQ, 0:CQ])
        nc.tensor.transpose(ps_wt[:, 32 : 32 + CQ], wk_sb[:], ident[0:CQ, 0:CQ])
        nc.tensor.transpose(ps_wt[:, 64 : 64 + CV], wv_sb[:], ident[0:CV, 0:CV])
        wcatT = const.tile([128, 128], BF16, tag="wcatT")
        nc.vector.memset(wcatT[:], 0.0)
        nc.vector.tensor_copy(wcatT[:, 0:CQ], ps_wt[:, 0:CQ])
        nc.vector.tensor_copy(wcatT[:, 32 : 32 + CQ], ps_wt[:, 32 : 32 + CQ])
        nc.vector.tensor_copy(wcatT[:, 64 : 64 + CV], ps_wt[:, 64 : 64 + CV])

        # wo^T [64, 128] bf16 (lhsT for output projection)
        ps_wo = ps_w.tile([128, NCH], FP32, tag="wm")
        nc.tensor.transpose(ps_wo[0:CV, 0:C], wo_sb[:], ident[:])
        woT = const.tile([CV, C], BF16, tag="woT")
        nc.vector.tensor_copy(woT[:], ps_wo[0:CV, 0:C])

        # ---------------- per-batch pipeline ----------------
        state = {}

        def load_x(b):
            x_sb = xpool.tile([C, HW], FP32, tag="x", name=f"x_{b}")
            x_bf = qkv.tile([C, HW], BF16, tag="xbf", bufs=2, name=f"xbf_{b}")
            # issue on the (idle at startup) ACT sequencer: the sync
            # sequencer is the kernel's scarcest descriptor-issue engine.
            # First chunks are split across queues so the projection
            # pipeline starts within a few us.
            if b == 0:
                for cc in range(2):
                    csl = slice(cc * NCH, (cc + 1) * NCH)
                    for s in range(4):
                        nc.scalar.dma_start(
                            x_sb[32 * s : 32 * (s + 1), csl],
                            x_e[b, 32 * s : 32 * (s + 1), csl],
                        )
                for cc in range(2, NCHUNKS):
                    csl = slice(cc * NCH, (cc + 1) * NCH)
                    nc.scalar.dma_start(x_sb[:, csl], x_e[b, :, csl])
            else:
                for qq in range(4):
                    csl = slice(qq * HW // 4, (qq + 1) * HW // 4)
                    nc.scalar.dma_start(x_sb[:, csl], x_e[b, :, csl])
            for qq in range(4):
                csl = slice(qq * HW // 4, (qq + 1) * HW // 4)
                # casting DMA f32->bf16, SBUF->SBUF (gpsimd-only feature);
                # avoids reading x from HBM twice
                nc.gpsimd.dma_start(x_bf[:, csl], x_sb[:, csl])
            return x_sb, x_bf

        def prep(b, x_sb, x_bf):
            st = {}
            # qkv_full: projection output staging; q lives at partitions
            # 0:16 (= packing strip 0), kpre at 32:48, vpre at 64:128.
            # Pooled k+v land in kv_sb at the SAME partition rows, so both
            # pool in a single pair of DVE ops per chunk (cost is free-size
            # bound, the extra partitions ride free).
            qkv_full = qkv.tile([C, HW], BF16, tag="qkvfull", name=f"qf_{b}")
            q_rep = qkv.tile([128, HW], BF16, tag="qrep", name=f"qr_{b}")
            kv_sb = qkv.tile([128, M], BF16, tag="k", name=f"kv_{b}")

            for cc in range(NCHUNKS):
                sl = slice(cc * NCH, (cc + 1) * NCH)
                ps_p = (ps_m if cc % 2 == 0 else ps_w).tile(
                    [128, NCH], FP32, tag="wm",
                    name=f"pj_{b}_{cc}",
                )
                nc.tensor.matmul(
                    ps_p[:], wcatT[:], x_bf[:, sl], start=True, stop=True
                )
                # one PSUM->SBUF drain (only one PSUM read operand is legal
                # per DVE op, so pooling must run from SBUF)
                nc.vector.tensor_copy(qkv_full[:, sl], ps_p[:])
                # maxpool 2x2: h-pairs first (contiguous last dim), then
                # w-pairs; engine APs at base!=0 cannot span >their 32/64
                # block, so k (rows 32:48) and v (rows 64:128) pool separately
                kv1 = qkv.tile([128, 4 * 64], BF16, tag="kv1", name=f"kv1_{b}_{cc}")
                for lo, hi in ((32, 32 + CQ), (64, 128)):
                    pp = qkv_full[lo:hi, sl].rearrange(
                        "p (h2 two w) -> p h2 two w", h2=4, two=2, w=64
                    )
                    s1 = kv1[lo:hi, :].rearrange("p (h w) -> p h w", h=4, w=64)
                    nc.vector.tensor_tensor(
                        s1, pp[:, :, 0, :], pp[:, :, 1, :], AluOpType.max
                    )
                    s1w = kv1[lo:hi, :].rearrange(
                        "p (h w2 two) -> p h w2 two", h=4, w2=32, two=2
                    )
                    s2 = kv_sb[lo:hi, cc * 128 : (cc + 1) * 128].rearrange(
                        "p (h w2) -> p h w2", h=4, w2=32
                    )
                    nc.vector.tensor_tensor(
                        s2, s1w[:, :, :, 0], s1w[:, :, :, 1], AluOpType.max
                    )

            # replicate q (per half-of-chunk-range) and k (per chunk) to
            # the other packing strips; pooled k already sits at kv_sb rows
            # 32:48 (= strip 1); other strips go into k_rep (rows 64:128 of
            # kv_sb hold v). Chunk granularity so scores start early.
            k_rep = qkv.tile([128, M], BF16, tag="krep", name=f"kr_{b}")
            if PACK:
                for h in range(2):
                    kh = slice(h * M // 2, (h + 1) * M // 2)
                    for s in (0, 2, 3):
                        nc.sync.dma_start(
                            k_rep[32 * s : 32 * s + CQ, kh],
                            kv_sb[32 : 32 + CQ, kh],
                        )
                    hsl = slice(h * EXPSPAN, (h + 1) * EXPSPAN)
                    for s in range(1, 4):
                        nc.sync.dma_start(
                            q_rep[32 * s : 32 * s + CQ, hsl],
                            qkv_full[0:CQ, hsl],
                        )
            else:
                nc.sync.dma_start(
                    k_rep[0:CQ, :], kv_sb[32 : 32 + CQ, :]
                )

            st.update(x_sb=x_sb, qkv_full=qkv_full, q_rep=q_rep, k_sb=kv_sb,
                      k_rep=k_rep, vT=[None] * MT, b=b)
            return st

        def emit_vt(st, j):
            # vT~ strip j: [128, 65] bf16, col 64 = ones (v lives at rows
            # 64:128 of kv_sb; identity block rows 64:128 matches its base).
            # Emitted lazily (interleaved into the first scores half) so the
            # in-order PE stream does not gate scores behind transposes.
            b, kv_sb = st["b"], st["k_sb"]
            ps_t = ps_w.tile([128, NCH * 2], BF16, tag="wm", name=f"tp_{b}_{j}")
            nc.tensor.transpose(
                ps_t[:, 0:CV],
                kv_sb[64:128, j * 128 : (j + 1) * 128],
                ident_bf[64:128, 64:128],
            )
            vt = vtpool.tile([128, CV + 1], BF16, tag="vt", name=f"vt_{b}_{j}")
            nc.vector.tensor_copy(vt[:, 0:CV], ps_t[:, 0:CV])
            nc.vector.memset(vt[:, CV : CV + 1], 1.0)
            st["vT"][j] = vt

        def scores_half(b, h, st, interleave):
            qkv_full, q_rep, k_sb = st["qkv_full"], st["q_rep"], st["k_sb"]
            k_rep = st["k_rep"]
            # scores + exp: 4-way row-group packed matmuls; two 2-bank
            # score buffers per m-tile so exp double-buffers against PE
            p_tiles = [
                ppool.tile([128, EXPSPAN], BF16, tag="p", name=f"p_{b}_{t}_{h}")
                for t in range(MT)
            ]
            for t in range(MT):
                interleave(t)
                sA = ps_s.tile([128, 2 * NCH], FP32, tag="s", name=f"sA_{b}_{h}_{t}")
                sB = ps_s.tile([128, 2 * NCH], FP32, tag="s", name=f"sB_{b}_{h}_{t}")
                tiles = [sA, sA, sB, sB]
                if PACK:
                    if True:
                        for i in range(4):
                            ncol = h * EXPSPAN + i * NCH
                            qsrc = qkv_full if i == 0 else q_rep
                            ksrc = k_sb if i == 1 else k_rep
                            nc.tensor.matmul(
                                tiles[i][:, (i % 2) * NCH : (i % 2 + 1) * NCH],
                                ksrc[
                                    32 * i : 32 * i + CQ,
                                    t * 128 : (t + 1) * 128,
                                ],
                                qsrc[32 * i : 32 * i + CQ, ncol : ncol + NCH],
                                start=True,
                                stop=True,
                                tile_position=(32 * i, 0),
                            )
                else:
                    for i in range(4):
                        ncol = h * EXPSPAN + i * NCH
                        nc.tensor.matmul(
                            tiles[i][:, (i % 2) * NCH : (i % 2 + 1) * NCH],
                            k_rep[0:CQ, t * 128 : (t + 1) * 128],
                            qkv_full[0:CQ, ncol : ncol + NCH],
                            start=True,
                            stop=True,
                        )
                nc.scalar.activation(
                    p_tiles[t][:, 0 : 2 * NCH], sA[:], AFT.Exp
                )
                nc.scalar.activation(
                    p_tiles[t][:, 2 * NCH : 4 * NCH], sB[:], AFT.Exp
                )

            return p_tiles

        def av_piece(b, h, st, p_tiles, ep, c, piece):
            vT = st["vT"]
            if piece == 0:
                ep["avps"][c] = ps_av.tile(
                    [128, NCH], FP32, tag="av", name=f"av_{b}_{h}_{c}"
                )
            o_ps = ep["avps"][c]
            for t in range(piece * MT // 2, (piece + 1) * MT // 2):
                nc.tensor.matmul(
                    o_ps[0 : CV + 1, :],
                    vT[t][:],
                    p_tiles[t][:, c * NCH : (c + 1) * NCH],
                    start=(t == 0),
                    stop=(t == MT - 1),
                )
            if piece == 1:
                # single drain: rows 0:64 = unnormalized AV, row 64 = den
                ou = eppool.tile([CV + 1, NCH], BF16, tag="ou", bufs=10,
                                 name=f"ou_{b}_{h}_{c}")
                nc.vector.tensor_copy(ou[:], o_ps[0 : CV + 1, :])
                dtile, drow = ep["dmap"](c)
                nc.sync.dma_start(dtile[drow : drow + 1, :], ou[CV : CV + 1, :])
                ep["ou"].append(ou)

        def av_chunk(b, h, st, p_tiles, ep, c):
            av_piece(b, h, st, p_tiles, ep, c, 0)
            av_piece(b, h, st, p_tiles, ep, c, 1)

        def den_chain(b, h, ep, c0=0, c1=4):
            # one packed reciprocal for the half's denominators [c0:c1]
            # (the dstage tile for this range always starts at partition 0)
            n = c1 - c0
            dtile, _ = ep["dmap"](c0)
            rdn = eppool.tile([4, NCH], FP32, tag="rdn", bufs=2,
                              name=f"rdn_{b}_{h}_{c0}")
            nc.vector.reciprocal(rdn[0:n, :], dtile[0:n, :])
            if "rd4" not in ep:
                ep["rd4"] = dscratch.tile([4, NCH], FP32, tag="rd",
                                          name=f"rd4_{b}_{h}")
            nc.sync.dma_start(ep["rd4"][c0:c1, :], rdn[0:n, :])

        def epilogue_chunk(b, h, st, ep, c):
            x_sb = st["x_sb"]
            cc = h * 4 + c
            sl = slice(cc * NCH, (cc + 1) * NCH)
            if "den" not in ep:
                ep["den"] = eppool.tile([CV, 4 * NCH], FP32, tag="den", bufs=2,
                                        name=f"den_{b}_{h}")
                ep["denq"] = set()
            q = c // 2
            if q not in ep["denq"]:
                ep["denq"].add(q)
                den = ep["den"]
                nc.sync.dma_start(
                    den[:, q * 2 * NCH : (q + 1) * 2 * NCH].rearrange(
                        "p (c n) -> p c n", c=2, n=NCH
                    ),
                    ep["rd4"][2 * q : 2 * q + 2, :].partition_broadcast(CV),
                )
            onorm = eppool.tile([CV, NCH], BF16, tag="onorm", bufs=3,
                                name=f"on_{b}_{h}_{c}")
            nc.vector.scalar_tensor_tensor(
                onorm[:],
                ep["ou"][c][0:CV, :],
                g_sb[0:CV, 0:1],
                ep["den"][:, c * NCH : (c + 1) * NCH],
                AluOpType.mult,
                AluOpType.mult,
            )
            o2_ps = ps_w.tile([128, NCH], FP32, tag="wm", name=f"o2_{b}_{h}_{c}")
            nc.tensor.matmul(o2_ps[:], woT[:], onorm[:], start=True, stop=True)
            out_sb = outpool.tile([C, NCH], FP32, tag="out",
                                  name=f"os_{b}_{h}_{c}")
            nc.vector.tensor_tensor(
                out_sb[:], o2_ps[:], x_sb[:, sl], AluOpType.add
            )
            nc.gpsimd.dma_start(out_e[b, :, sl], out_sb[:])

        # software-pipelined emission: prep(b+1) is emitted between the two
        # halves of batch b so its DMA/DVE/PE work fills batch b's exp phase
        # software-pipelined emission: the previous half's AV chunks are
        # emitted between the current half's score m-tiles so the PE stream
        # has work while exp drains each score buffer; prep(1) is emitted
        # after the first half so its work fills batch 0's exp phase.
        x0 = load_x(0)
        x1 = load_x(1)
        states = {0: prep(0, *x0)}
        seq = [(0, 0), (0, 1), (1, 0), (1, 1)]
        prev = None

        def make_interleave(cur_st, cur_h, prev, prev2):
            def hook(t):
                if cur_h == 0 and cur_st["vT"][t] is None:
                    emit_vt(cur_st, t)
                if prev is not None:
                    pb, ph, pst, ppt, pep = prev
                    av_piece(pb, ph, pst, ppt, pep, t // 2, t % 2)
                    if t == 7:
                        den_chain(pb, ph, pep)
                if prev2 is not None and t in (0, 2, 4, 6):
                    # epilogue of the half before last: its rden round-trip
                    # is long since complete, so these never stall PE
                    epilogue_chunk(prev2[0], prev2[1], prev2[2], prev2[4], t // 2)
            return hook

        def make_ep(b, h, split):
            if split:
                tiles = [
                    eppool.tile([2, NCH], BF16, tag="dstage", bufs=3,
                                name=f"dst_{b}_{h}_{q}")
                    for q in range(2)
                ]
                return {"dmap": lambda c: (tiles[c // 2], c % 2), "ou": [], "avps": {}}
            tile = eppool.tile([4, NCH], BF16, tag="dstage", bufs=3,
                               name=f"dst_{b}_{h}")
            return {"dmap": lambda c: (tile, c), "ou": [], "avps": {}}

        prev2 = None
        for idx, (b, h) in enumerate(seq):
            st = states[b]
            ep = make_ep(b, h, (b, h) == (1, 1))
            p_tiles = scores_half(b, h, st, make_interleave(st, h, prev, prev2))
            if idx == 0:
                states[1] = prep(1, *x1)
            prev2 = prev
            prev = (b, h, st, p_tiles, ep)

        # drain the two in-flight halves; the final half goes quarter by
        # quarter so its epilogue chains start as early as possible
        for c in range(4):
            epilogue_chunk(prev2[0], prev2[1], prev2[2], prev2[4], c)
        for c in range(2):
            av_chunk(prev[0], prev[1], prev[2], prev[3], prev[4], c)
        den_chain(prev[0], prev[1], prev[4], 0, 2)
        for c in range(2):
            av_chunk(prev[0], prev[1], prev[2], prev[3], prev[4], 2 + c)
        for c in range(2):
            epilogue_chunk(prev[0], prev[1], prev[2], prev[4], c)
        den_chain(prev[0], prev[1], prev[4], 2, 4)
        for c in range(2):
            epilogue_chunk(prev[0], prev[1], prev[2], prev[4], 2 + c)

    nc.finalize()
    return nc


_NC_CACHE = None


def _get_nc():
    global _NC_CACHE
    if _NC_CACHE is None:
        _NC_CACHE = build_nc()
    return _NC_CACHE


def kernel(**inputs) -> np.ndarray:
    from concourse.bass_utils import run_bass_kernel_spmd

    x = np.asarray(inputs["x"], dtype=np.float32).reshape(B_FULL, C, HW)
    wq = np.asarray(inputs["wq"], dtype=np.float32)
    wk = np.asarray(inputs["wk"], dtype=np.float32)
    wv = np.asarray(inputs["wv"], dtype=np.float32)
    wo = np.asarray(inputs["wo"], dtype=np.float32)
    gamma = np.asarray(inputs["gamma"], dtype=np.float32)

    nc = _get_nc()
    in_maps = []
    for i in range(N_CORES):
        in_maps.append(
            {
                "x": np.ascontiguousarray(x[i * B_LOC : (i + 1) * B_LOC]),
                "wq": wq,
                "wk": wk,
                "wv": wv,
                "wo": wo,
                "gamma": gamma,
            }
        )
    res = run_bass_kernel_spmd(nc, in_maps, core_ids=list(range(N_CORES)))
    outs = [res.results[i]["out"].reshape(B_LOC, C, H, W) for i in range(N_CORES)]
    return np.concatenate(outs, axis=0)


if __name__ == "__main__":
    import reference

    inputs = {k: np.asarray(v) for k, v in reference.setup_inputs().items()}
    expected = np.asarray(reference.reference(**inputs))
    actual = kernel(**inputs)
    err = np.linalg.norm(actual - expected) / np.linalg.norm(expected)
    print("Relative error:", err)


# revision 41
# speedup vs baseline: 1.0695x; 1.0695x over previous
"""Trainium2 Bass kernel for nn_Attention_17334488007364.

Computation (per batch element, x as [C=128, N=4096]):
    q = wq @ x                      [16, 4096]
    k = maxpool2(wk @ x)            [16, 1024]
    v = maxpool2(wv @ x)            [64, 1024]
    attn = softmax(q^T k, axis=m)   [4096, 1024]
    o = v @ attn^T                  [64, 4096]
    out = gamma * (wo @ o) + x      [128, 4096]

Sharding: pure data parallel — B=16 over 8 cores, 2 batch elements/core.

Per-core dataflow ("transposed" layout, m on partitions everywhere):
  - qkv projection fused into one bf16 matmul; x is cast f32->bf16 by a
    gpsimd DMA straight from DRAM (gpsimd DMAs can cast)
  - scores^T[m_tile][128, n] = k_tile^T q  (bf16, K=16; 4-way row-group
    packing via tile_position so 4 n-chunks compute concurrently)
  - p = exp(scores) on ACT straight out of PSUM in [128, 2048] calls
    (ACT is the bottleneck: 8.4M exps/core ~= 55us floor)
  - o_u[65, n-chunk] accumulates sum_m vT~[128,65].T @ p where vT~ has a
    trailing ones column, so row 64 = softmax denominator for free
  - denominators of 4 chunks are packed to partitions 0..3 via DMA and
    inverted by ONE DVE reciprocal per half (native recip cost is mostly
    per-call, so packing is ~4x cheaper)
  - 1/den broadcast to 64 partitions via a DRAM round-trip DMA; then
    onorm = (ou * gamma) * rden in one scalar_tensor_tensor, wo matmul,
    and residual out = o2 + x in one tensor_tensor.
"""

from contextlib import ExitStack

import numpy as np

import concourse.bacc as bacc
import concourse.mybir as mybir
from concourse import masks
from concourse.alu_op_type import AluOpType
from concourse.tile import TileContext

FP32 = mybir.dt.float32
BF16 = mybir.dt.bfloat16
AFT = mybir.ActivationFunctionType

# Per-core problem shape (hardcoded; harness provides full inputs).
B_FULL, C, H, W = 16, 128, 64, 64
N_CORES = 8
B_LOC = B_FULL // N_CORES            # 2
HW = H * W                           # 4096
M = HW // 4                          # 1024 (after 2x2 maxpool)
CQ, CV = C // 8, C // 2              # 16, 64
NCH = 512                            # psum-bank-sized n chunk
NCHUNKS = HW // NCH                  # 8
EXPSPAN = 2048                       # 4 banks per exp call
MT = M // 128                        # 8 m-tiles of 128

PACK = True                          # 4-way row-group packing for scores


def build_nc():
    nc = bacc.Bacc()
    x_e = nc.declare_dram_parameter("x", [B_LOC, C, HW], FP32, isOutput=False)
    wq_e = nc.declare_dram_parameter("wq", [CQ, C], FP32, isOutput=False)
    wk_e = nc.declare_dram_parameter("wk", [CQ, C], FP32, isOutput=False)
    wv_e = nc.declare_dram_parameter("wv", [CV, C], FP32, isOutput=False)
    wo_e = nc.declare_dram_parameter("wo", [C, CV], FP32, isOutput=False)
    g_e = nc.declare_dram_parameter("gamma", [1], FP32, isOutput=False)
    out_e = nc.declare_dram_parameter("out", [B_LOC, C, HW], FP32, isOutput=True)

    with TileContext(nc) as tc, ExitStack() as ctx:
        const = ctx.enter_context(tc.tile_pool(name="const", bufs=1))
        xpool = ctx.enter_context(tc.tile_pool(name="x", bufs=2))
        qkv = ctx.enter_context(tc.tile_pool(name="qkv", bufs=2))
        ppool = ctx.enter_context(tc.tile_pool(name="p", bufs=17))
        vtpool = ctx.enter_context(tc.tile_pool(name="vt", bufs=18))
        eppool = ctx.enter_context(tc.tile_pool(name="ep", bufs=3))
        outpool = ctx.enter_context(tc.tile_pool(name="outp", bufs=3))
        # PSUM budget (8 banks): scores 4 + av 2 + w 1 + m 1
        ps_s = ctx.enter_context(tc.tile_pool(name="ps_s", bufs=3, space="PSUM"))
        ps_av = ctx.enter_context(tc.tile_pool(name="ps_av", bufs=1, space="PSUM"))
        ps_w = ctx.enter_context(tc.tile_pool(name="ps_w", bufs=1, space="PSUM"))
        ps_m = ps_w
        dscratch = ctx.enter_context(tc.tile_pool(name="dscr", bufs=4, space="DRAM"))

        # ---------------- constants / weight preprocessing ----------------
        ident = const.tile([128, 128], FP32)
        masks.make_identity(nc, ident[:])
        ident_bf = const.tile([128, 128], BF16)
        masks.make_identity(nc, ident_bf[:])

        wq_sb = const.tile([CQ, C], FP32, tag="wq")
        wk_sb = const.tile([CQ, C], FP32, tag="wk")
        wv_sb = const.tile([CV, C], FP32, tag="wv")
        wo_sb = const.tile([C, CV], FP32, tag="wo")
        nc.sync.dma_start(wq_sb[:], wq_e[:])
        nc.sync.dma_start(wk_sb[:], wk_e[:])
        nc.sync.dma_start(wv_sb[:], wv_e[:])
        nc.sync.dma_start(wo_sb[:], wo_e[:])

        # gamma broadcast to all 128 partitions: [128, 1]
        g_sb = const.tile([128, 1], FP32, tag="g")
        nc.gpsimd.dma_start(
            g_sb[:, 0:1], g_e[:].unsqueeze(0).partition_broadcast(128)
        )

        # W_cat^T: cols 0:16 = wq^T, 32:48 = wk^T, 64:128 = wv^T (32-aligned
        # so PSUM consumer slices start at partition 0/32/64)
        ps_wt = ps_w.tile([128, NCH], FP32, tag="wm")
        nc.tensor.transpose(ps_wt[:, 0:CQ], wq_sb[:], ident[0:CQ, 0:CQ])
        nc.tensor.transpose(ps_wt[:, 32 : 32 + CQ], wk_sb[:], ident[0:CQ, 0:CQ])
        nc.tensor.transpose(ps_wt[:, 64 : 64 + CV], wv_sb[:], ident[0:CV, 0:CV])
        wcatT = const.tile([128, 128], BF16, tag="wcatT")
        nc.vector.memset(wcatT[:], 0.0)
        nc.vector.tensor_copy(wcatT[:, 0:CQ], ps_wt[:, 0:CQ])
        nc.vector.tensor_copy(wcatT[:, 32 : 32 + CQ], ps_wt[:, 32 : 32 + CQ])
        nc.vector.tensor_copy(wcatT[:, 64 : 64 + CV], ps_wt[:, 64 : 64 + CV])

        # wo^T [64, 128] bf16 (lhsT for output projection)
        ps_wo = ps_w.tile([128, NCH], FP32, tag="wm")
        nc.tensor.transpose(ps_wo[0:CV, 0:C], wo_sb[:], ident[:])
        woT = const.tile([CV, C], BF16, tag="woT")
        nc.vector.tensor_copy(woT[:], ps_wo[0:CV, 0:C])

        # ---------------- per-batch pipeline ----------------
        state = {}

        def load_x(b):
            x_sb = xpool.tile([C, HW], FP32, tag="x", name=f"x_{b}")
            x_bf = qkv.tile([C, HW], BF16, tag="xbf", bufs=2, name=f"xbf_{b}")
            # issue on the (idle at startup) ACT sequencer: the sync
            # sequencer is the kernel's scarcest descriptor-issue engine.
            # First chunks are split across queues so the projection
            # pipeline starts within a few us.
            if b == 0:
                for cc in range(2):
                    csl = slice(cc * NCH, (cc + 1) * NCH)
                    for s in range(4):
                        nc.scalar.dma_start(
                            x_sb[32 * s : 32 * (s + 1), csl],
                            x_e[b, 32 * s : 32 * (s + 1), csl],
                        )
                for cc in range(2, NCHUNKS):
                    csl = slice(cc * NCH, (cc + 1) * NCH)
                    nc.scalar.dma_start(x_sb[:, csl], x_e[b, :, csl])
            else:
                for qq in range(4):
                    csl = slice(qq * HW // 4, (qq + 1) * HW // 4)
                    nc.scalar.dma_start(x_sb[:, csl], x_e[b, :, csl])
            for qq in range(4):
                csl = slice(qq * HW // 4, (qq + 1) * HW // 4)
                # casting DMA f32->bf16, SBUF->SBUF (gpsimd-only feature);
                # avoids reading x from HBM twice
                nc.gpsimd.dma_start(x_bf[:, csl], x_sb[:, csl])
            return x_sb, x_bf

        def prep(b, x_sb, x_bf):
            st = {}
            # qkv_full: projection output staging; q lives at partitions
            # 0:16 (= packing strip 0), kpre at 32:48, vpre at 64:128.
            # Pooled k+v land in kv_sb at the SAME partition rows, so both
            # pool in a single pair of DVE ops per chunk (cost is free-size
            # bound, the extra partitions ride free).
            qkv_full = qkv.tile([C, HW], BF16, tag="qkvfull", name=f"qf_{b}")
            q_rep = qkv.tile([128, HW], BF16, tag="qrep", name=f"qr_{b}")
            kv_sb = qkv.tile([128, M], BF16, tag="k", name=f"kv_{b}")

            for cc in range(NCHUNKS):
                sl = slice(cc * NCH, (cc + 1) * NCH)
                ps_p = (ps_m if cc % 2 == 0 else ps_w).tile(
                    [128, NCH], FP32, tag="wm",
                    name=f"pj_{b}_{cc}",
                )
                nc.tensor.matmul(
                    ps_p[:], wcatT[:], x_bf[:, sl], start=True, stop=True
                )
                # one PSUM->SBUF drain (only one PSUM read operand is legal
                # per DVE op, so pooling must run from SBUF)
                nc.vector.tensor_copy(qkv_full[:, sl], ps_p[:])
                # maxpool 2x2: h-pairs first (contiguous last dim), then
                # w-pairs; engine APs at base!=0 cannot span >their 32/64
                # block, so k (rows 32:48) and v (rows 64:128) pool separately
                kv1 = qkv.tile([128, 4 * 64], BF16, tag="kv1", name=f"kv1_{b}_{cc}")
                for lo, hi in ((32, 32 + CQ), (64, 128)):
                    pp = qkv_full[lo:hi, sl].rearrange(
                        "p (h2 two w) -> p h2 two w", h2=4, two=2, w=64
                    )
                    s1 = kv1[lo:hi, :].rearrange("p (h w) -> p h w", h=4, w=64)
                    nc.vector.tensor_tensor(
                        s1, pp[:, :, 0, :], pp[:, :, 1, :], AluOpType.max
                    )
                    s1w = kv1[lo:hi, :].rearrange(
                        "p (h w2 two) -> p h w2 two", h=4, w2=32, two=2
                    )
                    s2 = kv_sb[lo:hi, cc * 128 : (cc + 1) * 128].rearrange(
                        "p (h w2) -> p h w2", h=4, w2=32
                    )
                    nc.vector.tensor_tensor(
                        s2, s1w[:, :, :, 0], s1w[:, :, :, 1], AluOpType.max
                    )

            # replicate q (per half-of-chunk-range) and k (per chunk) to
            # the other packing strips; pooled k already sits at kv_sb rows
            # 32:48 (= strip 1); other strips go into k_rep (rows 64:128 of
            # kv_sb hold v). Chunk granularity so scores start early.
            k_rep = qkv.tile([128, M], BF16, tag="krep", name=f"kr_{b}")
            if PACK:
                for h in range(2):
                    kh = slice(h * M // 2, (h + 1) * M // 2)
                    for s in (0, 2, 3):
                        nc.sync.dma_start(
                            k_rep[32 * s : 32 * s + CQ, kh],
                            kv_sb[32 : 32 + CQ, kh],
                        )
                    hsl = slice(h * EXPSPAN, (h + 1) * EXPSPAN)
                    for s in range(1, 4):
                        nc.sync.dma_start(
                            q_rep[32 * s : 32 * s + CQ, hsl],
                            qkv_full[0:CQ, hsl],
                        )
            else:
                nc.sync.dma_start(
                    k_rep[0:CQ, :], kv_sb[32 : 32 + CQ, :]
                )

            st.update(x_sb=x_sb, qkv_full=qkv_full, q_rep=q_rep, k_sb=kv_sb,
                      k_rep=k_rep, vT=[None] * MT, b=b)
            return st

        def emit_vt(st, j):
            # vT~ strip j: [128, 65] bf16, col 64 = ones (v lives at rows
            # 64:128 of kv_sb; identity block rows 64:128 matches its base).
            # Emitted lazily (interleaved into the first scores half) so the
            # in-order PE stream does not gate scores behind transposes.
            b, kv_sb = st["b"], st["k_sb"]
            ps_t = ps_w.tile([128, NCH * 2], BF16, tag="wm", name=f"tp_{b}_{j}")
            nc.tensor.transpose(
                ps_t[:, 0:CV],
                kv_sb[64:128, j * 128 : (j + 1) * 128],
                ident_bf[64:128, 64:128],
            )
            vt = vtpool.tile([128, CV + 1], BF16, tag="vt", name=f"vt_{b}_{j}")
            nc.vector.tensor_copy(vt[:, 0:CV], ps_t[:, 0:CV])
            nc.vector.memset(vt[:, CV : CV + 1], 1.0)
            st["vT"][j] = vt

        def scores_half(b, h, st, interleave):
            qkv_full, q_rep, k_sb = st["qkv_full"], st["q_rep"], st["k_sb"]
            k_rep = st["k_rep"]
            # scores + exp: 4-way row-group packed matmuls; two 2-bank
            # score buffers per m-tile so exp double-buffers against PE
            p_tiles = [
                ppool.tile([128, EXPSPAN], BF16, tag="p", name=f"p_{b}_{t}_{h}")
                for t in range(MT)
            ]
            for t in range(MT):
                interleave(t)
                sA = ps_s.tile([128, 2 * NCH], FP32, tag="s", name=f"sA_{b}_{h}_{t}")
                sB = ps_s.tile([128, 2 * NCH], FP32, tag="s", name=f"sB_{b}_{h}_{t}")
                tiles = [sA, sA, sB, sB]
                if PACK:
                    if True:
                        for i in range(4):
                            ncol = h * EXPSPAN + i * NCH
                            qsrc = qkv_full if i == 0 else q_rep
                            ksrc = k_sb if i == 1 else k_rep
                            nc.tensor.matmul(
                                tiles[i][:, (i % 2) * NCH : (i % 2 + 1) * NCH],
                                ksrc[
                                    32 * i : 32 * i + CQ,
                                    t * 128 : (t + 1) * 128,
                                ],
                                qsrc[32 * i : 32 * i + CQ, ncol : ncol + NCH],
                                start=True,
                                stop=True,
                                tile_position=(32 * i, 0),
                            )
                else:
                    for i in range(4):
                        ncol = h * EXPSPAN + i * NCH
                        nc.tensor.matmul(
                            tiles[i][:, (i % 2) * NCH : (i % 2 + 1) * NCH],
                            k_rep[0:CQ, t * 128 : (t + 1) * 128],
                            qkv_full[0:CQ, ncol : ncol + NCH],
                            start=True,
                            stop=True,
                        )
                nc.scalar.activation(
                    p_tiles[t][:, 0 : 2 * NCH], sA[:], AFT.Exp
                )
                nc.scalar.activation(
                    p_tiles[t][:, 2 * NCH : 4 * NCH], sB[:], AFT.Exp
                )

            return p_tiles

        def av_piece(b, h, st, p_tiles, ep, c, piece):
            vT = st["vT"]
            if piece == 0:
                ep["avps"][c] = ps_av.tile(
                    [128, NCH], FP32, tag="av", name=f"av_{b}_{h}_{c}"
                )
            o_ps = ep["avps"][c]
            for t in range(piece * MT // 2, (piece + 1) * MT // 2):
                nc.tensor.matmul(
                    o_ps[0 : CV + 1, :],
                    vT[t][:],
                    p_tiles[t][:, c * NCH : (c + 1) * NCH],
                    start=(t == 0),
                    stop=(t == MT - 1),
                )
            if piece == 1:
                # single drain: rows 0:64 = unnormalized AV, row 64 = den
                ou = eppool.tile([CV + 1, NCH], BF16, tag="ou", bufs=10,
                                 name=f"ou_{b}_{h}_{c}")
                nc.vector.tensor_copy(ou[:], o_ps[0 : CV + 1, :])
                dtile, drow = ep["dmap"](c)
                nc.sync.dma_start(dtile[drow : drow + 1, :], ou[CV : CV + 1, :])
                ep["ou"].append(ou)

        def av_chunk(b, h, st, p_tiles, ep, c):
            av_piece(b, h, st, p_tiles, ep, c, 0)
            av_piece(b, h, st, p_tiles, ep, c, 1)

        def den_chain(b, h, ep, c0=0, c1=4):
            # one packed reciprocal for the half's denominators [c0:c1]
            # (the dstage tile for this range always starts at partition 0)
            n = c1 - c0
            dtile, _ = ep["dmap"](c0)
            rdn = eppool.tile([4, NCH], FP32, tag="rdn", bufs=2,
                              name=f"rdn_{b}_{h}_{c0}")
            nc.vector.reciprocal(rdn[0:n, :], dtile[0:n, :])
            if "rd4" not in ep:
                ep["rd4"] = dscratch.tile([4, NCH], FP32, tag="rd",
                                          name=f"rd4_{b}_{h}")
            nc.sync.dma_start(ep["rd4"][c0:c1, :], rdn[0:n, :])

        def epilogue_chunk(b, h, st, ep, c):
            x_sb = st["x_sb"]
            cc = h * 4 + c
            sl = slice(cc * NCH, (cc + 1) * NCH)
            if "den" not in ep:
                ep["den"] = eppool.tile([CV, 4 * NCH], FP32, tag="den", bufs=2,
                                        name=f"den_{b}_{h}")
                ep["denq"] = set()
            q = c // 2
            if q not in ep["denq"]:
                ep["denq"].add(q)
                den = ep["den"]
                nc.sync.dma_start(
                    den[:, q * 2 * NCH : (q + 1) * 2 * NCH].rearrange(
                        "p (c n) -> p c n", c=2, n=NCH
                    ),
                    ep["rd4"][2 * q : 2 * q + 2, :].partition_broadcast(CV),
                )
            onorm = eppool.tile([CV, NCH], BF16, tag="onorm", bufs=3,
                                name=f"on_{b}_{h}_{c}")
            nc.vector.scalar_tensor_tensor(
                onorm[:],
                ep["ou"][c][0:CV, :],
                g_sb[0:CV, 0:1],
                ep["den"][:, c * NCH : (c + 1) * NCH],
                AluOpType.mult,
                AluOpType.mult,
            )
            o2_ps = ps_w.tile([128, NCH], FP32, tag="wm", name=f"o2_{b}_{h}_{c}")
            nc.tensor.matmul(o2_ps[:], woT[:], onorm[:], start=True, stop=True)
            out_sb = outpool.tile([C, NCH], FP32, tag="out",
                                  name=f"os_{b}_{h}_{c}")
            nc.vector.tensor_tensor(
                out_sb[:], o2_ps[:], x_sb[:, sl], AluOpType.add
            )
            nc.gpsimd.dma_start(out_e[b, :, sl], out_sb[:])

        # software-pipelined emission: prep(b+1) is emitted between the two
        # halves of batch b so its DMA/DVE/PE work fills batch b's exp phase
        # software-pipelined emission: the previous half's AV chunks are
        # emitted between the current half's score m-tiles so the PE stream
        # has work while exp drains each score buffer; prep(1) is emitted
        # after the first half so its work fills batch 0's exp phase.
        x0 = load_x(0)
        x1 = load_x(1)
        states = {0: prep(0, *x0)}
        seq = [(0, 0), (0, 1), (1, 0), (1, 1)]
        prev = None

        def make_interleave(cur_st, cur_h, prev, prev2):
            def hook(t):
                if cur_h == 0 and cur_st["vT"][t] is None:
                    emit_vt(cur_st, t)
                if prev is not None:
                    pb, ph, pst, ppt, pep = prev
                    if t in (1, 3, 5, 7):
                        av_chunk(pb, ph, pst, ppt, pep, (t - 1) // 2)
                    if t == 7:
                        den_chain(pb, ph, pep)
                if prev2 is not None and t in (0, 2, 4, 6):
                    # epilogue of the half before last: its rden round-trip
                    # is long since complete, so these never stall PE
                    epilogue_chunk(prev2[0], prev2[1], prev2[2], prev2[4], t // 2)
            return hook

        def make_ep(b, h, split):
            if split:
                tiles = [
                    eppool.tile([2, NCH], BF16, tag="dstage", bufs=3,
                                name=f"dst_{b}_{h}_{q}")
                    for q in range(2)
                ]
                return {"dmap": lambda c: (tiles[c // 2], c % 2), "ou": [], "avps": {}}
            tile = eppool.tile([4, NCH], BF16, tag="dstage", bufs=3,
                               name=f"dst_{b}_{h}")
            return {"dmap": lambda c: (tile, c), "ou": [], "avps": {}}

        prev2 = None
        for idx, (b, h) in enumerate(seq):
            st = states[b]
            ep = make_ep(b, h, (b, h) == (1, 1))
            p_tiles = scores_half(b, h, st, make_interleave(st, h, prev, prev2))
            if idx == 0:
                states[1] = prep(1, *x1)
            prev2 = prev
            prev = (b, h, st, p_tiles, ep)

        # drain the two in-flight halves; the final half goes quarter by
        # quarter so its epilogue chains start as early as possible
        for c in range(4):
            epilogue_chunk(prev2[0], prev2[1], prev2[2], prev2[4], c)
        for c in range(2):
            av_chunk(prev[0], prev[1], prev[2], prev[3], prev[4], c)
        den_chain(prev[0], prev[1], prev[4], 0, 2)
        for c in range(2):
            av_chunk(prev[0], prev[1], prev[2], prev[3], prev[4], 2 + c)
        for c in range(2):
            epilogue_chunk(prev[0], prev[1], prev[2], prev[4], c)
        den_chain(prev[0], prev[1], prev[4], 2, 4)
        for c in range(2):
            epilogue_chunk(prev[0], prev[1], prev[2], prev[4], 2 + c)

    nc.finalize()
    return nc


_NC_CACHE = None


def _get_nc():
    global _NC_CACHE
    if _NC_CACHE is None:
        _NC_CACHE = build_nc()
    return _NC_CACHE


def kernel(**inputs) -> np.ndarray:
    from concourse.bass_utils import run_bass_kernel_spmd

    x = np.asarray(inputs["x"], dtype=np.float32).reshape(B_FULL, C, HW)
    wq = np.asarray(inputs["wq"], dtype=np.float32)
    wk = np.asarray(inputs["wk"], dtype=np.float32)
    wv = np.asarray(inputs["wv"], dtype=np.float32)
    wo = np.asarray(inputs["wo"], dtype=np.float32)
    gamma = np.asarray(inputs["gamma"], dtype=np.float32)

    nc = _get_nc()
    in_maps = []
    for i in range(N_CORES):
        in_maps.append(
            {
                "x": np.ascontiguousarray(x[i * B_LOC : (i + 1) * B_LOC]),
                "wq": wq,
                "wk": wk,
                "wv": wv,
                "wo": wo,
                "gamma": gamma,
            }
        )
    res = run_bass_kernel_spmd(nc, in_maps, core_ids=list(range(N_CORES)))
    outs = [res.results[i]["out"].reshape(B_LOC, C, H, W) for i in range(N_CORES)]
    return np.concatenate(outs, axis=0)


if __name__ == "__main__":
    import reference

    inputs = {k: np.asarray(v) for k, v in reference.setup_inputs().items()}
    expected = np.asarray(reference.reference(**inputs))
    actual = kernel(**inputs)
    err = np.linalg.norm(actual - expected) / np.linalg.norm(expected)
    print("Relative error:", err)


# revision 42
# speedup vs baseline: 1.1637x; 1.0881x over previous
"""Trainium2 Bass kernel for nn_Attention_17334488007364.

Computation (per batch element, x as [C=128, N=4096]):
    q = wq @ x                      [16, 4096]
    k = maxpool2(wk @ x)            [16, 1024]
    v = maxpool2(wv @ x)            [64, 1024]
    attn = softmax(q^T k, axis=m)   [4096, 1024]
    o = v @ attn^T                  [64, 4096]
    out = gamma * (wo @ o) + x      [128, 4096]

Sharding: pure data parallel — B=16 over 8 cores, 2 batch elements/core.

Per-core dataflow ("transposed" layout, m on partitions everywhere):
  - qkv projection fused into one bf16 matmul; x is cast f32->bf16 by a
    gpsimd DMA straight from DRAM (gpsimd DMAs can cast)
  - scores^T[m_tile][128, n] = k_tile^T q  (bf16, K=16; 4-way row-group
    packing via tile_position so 4 n-chunks compute concurrently)
  - p = exp(scores) on ACT straight out of PSUM in [128, 2048] calls
    (ACT is the bottleneck: 8.4M exps/core ~= 55us floor)
  - o_u[65, n-chunk] accumulates sum_m vT~[128,65].T @ p where vT~ has a
    trailing ones column, so row 64 = softmax denominator for free
  - denominators of 4 chunks are packed to partitions 0..3 via DMA and
    inverted by ONE DVE reciprocal per half (native recip cost is mostly
    per-call, so packing is ~4x cheaper)
  - 1/den broadcast to 64 partitions via a DRAM round-trip DMA; then
    onorm = (ou * gamma) * rden in one scalar_tensor_tensor, wo matmul,
    and residual out = o2 + x in one tensor_tensor.
"""

from contextlib import ExitStack

import numpy as np

import concourse.bacc as bacc
import concourse.mybir as mybir
from concourse import masks
from concourse.alu_op_type import AluOpType
from concourse.tile import TileContext

FP32 = mybir.dt.float32
BF16 = mybir.dt.bfloat16
AFT = mybir.ActivationFunctionType

# Per-core problem shape (hardcoded; harness provides full inputs).
B_FULL, C, H, W = 16, 128, 64, 64
N_CORES = 8
B_LOC = B_FULL // N_CORES            # 2
HW = H * W                           # 4096
M = HW // 4                          # 1024 (after 2x2 maxpool)
CQ, CV = C // 8, C // 2              # 16, 64
NCH = 512                            # psum-bank-sized n chunk
NCHUNKS = HW // NCH                  # 8
EXPSPAN = 2048                       # 4 banks per exp call
MT = M // 128                        # 8 m-tiles of 128

PACK = True                          # 4-way row-group packing for scores


def build_nc():
    nc = bacc.Bacc()
    x_e = nc.declare_dram_parameter("x", [B_LOC, C, HW], FP32, isOutput=False)
    wq_e = nc.declare_dram_parameter("wq", [CQ, C], FP32, isOutput=False)
    wk_e = nc.declare_dram_parameter("wk", [CQ, C], FP32, isOutput=False)
    wv_e = nc.declare_dram_parameter("wv", [CV, C], FP32, isOutput=False)
    wo_e = nc.declare_dram_parameter("wo", [C, CV], FP32, isOutput=False)
    g_e = nc.declare_dram_parameter("gamma", [1], FP32, isOutput=False)
    out_e = nc.declare_dram_parameter("out", [B_LOC, C, HW], FP32, isOutput=True)

    with TileContext(nc) as tc, ExitStack() as ctx:
        const = ctx.enter_context(tc.tile_pool(name="const", bufs=1))
        xpool = ctx.enter_context(tc.tile_pool(name="x", bufs=2))
        qkv = ctx.enter_context(tc.tile_pool(name="qkv", bufs=2))
        ppool = ctx.enter_context(tc.tile_pool(name="p", bufs=17))
        vtpool = ctx.enter_context(tc.tile_pool(name="vt", bufs=18))
        eppool = ctx.enter_context(tc.tile_pool(name="ep", bufs=3))
        outpool = ctx.enter_context(tc.tile_pool(name="outp", bufs=3))
        # PSUM budget (8 banks): scores 4 + av 2 + w 1 + m 1
        ps_s = ctx.enter_context(tc.tile_pool(name="ps_s", bufs=3, space="PSUM"))
        ps_av = ctx.enter_context(tc.tile_pool(name="ps_av", bufs=1, space="PSUM"))
        ps_w = ctx.enter_context(tc.tile_pool(name="ps_w", bufs=1, space="PSUM"))
        ps_m = ps_w
        dscratch = ctx.enter_context(tc.tile_pool(name="dscr", bufs=4, space="DRAM"))

        # ---------------- constants / weight preprocessing ----------------
        ident = const.tile([128, 128], FP32)
        masks.make_identity(nc, ident[:])
        ident_bf = const.tile([128, 128], BF16)
        masks.make_identity(nc, ident_bf[:])

        wq_sb = const.tile([CQ, C], FP32, tag="wq")
        wk_sb = const.tile([CQ, C], FP32, tag="wk")
        wv_sb = const.tile([CV, C], FP32, tag="wv")
        wo_sb = const.tile([C, CV], FP32, tag="wo")
        nc.sync.dma_start(wq_sb[:], wq_e[:])
        nc.sync.dma_start(wk_sb[:], wk_e[:])
        nc.sync.dma_start(wv_sb[:], wv_e[:])
        nc.sync.dma_start(wo_sb[:], wo_e[:])

        # gamma broadcast to all 128 partitions: [128, 1]
        g_sb = const.tile([128, 1], FP32, tag="g")
        nc.gpsimd.dma_start(
            g_sb[:, 0:1], g_e[:].unsqueeze(0).partition_broadcast(128)
        )

        # W_cat^T: cols 0:16 = wq^T, 32:48 = wk^T, 64:128 = wv^T (32-aligned
        # so PSUM consumer slices start at partition 0/32/64)
        ps_wt = ps_w.tile([128, NCH], FP32, tag="wm")
        nc.tensor.transpose(ps_wt[:, 0:CQ], wq_sb[:], ident[0:CQ, 0:CQ])
        nc.tensor.transpose(ps_wt[:, 32 : 32 + CQ], wk_sb[:], ident[0:CQ, 0:CQ])
        nc.tensor.transpose(ps_wt[:, 64 : 64 + CV], wv_sb[:], ident[0:CV, 0:CV])
        wcatT = const.tile([128, 128], BF16, tag="wcatT")
        nc.vector.memset(wcatT[:], 0.0)
        nc.vector.tensor_copy(wcatT[:, 0:CQ], ps_wt[:, 0:CQ])
        nc.vector.tensor_copy(wcatT[:, 32 : 32 + CQ], ps_wt[:, 32 : 32 + CQ])
        nc.vector.tensor_copy(wcatT[:, 64 : 64 + CV], ps_wt[:, 64 : 64 + CV])

        # wo^T [64, 128] bf16 (lhsT for output projection)
        ps_wo = ps_w.tile([128, NCH], FP32, tag="wm")
        nc.tensor.transpose(ps_wo[0:CV, 0:C], wo_sb[:], ident[:])
        woT = const.tile([CV, C], BF16, tag="woT")
        nc.vector.tensor_copy(woT[:], ps_wo[0:CV, 0:C])

        # ---------------- per-batch pipeline ----------------
        state = {}

        def load_x(b):
            x_sb = xpool.tile([C, HW], FP32, tag="x", name=f"x_{b}")
            x_bf = qkv.tile([C, HW], BF16, tag="xbf", bufs=2, name=f"xbf_{b}")
            for qq in range(4):
                csl = slice(qq * HW // 4, (qq + 1) * HW // 4)
                # issue on the (idle at startup) ACT sequencer: the sync
                # sequencer is the kernel's scarcest descriptor-issue engine
                nc.scalar.dma_start(x_sb[:, csl], x_e[b, :, csl])
                # casting DMA f32->bf16, SBUF->SBUF (gpsimd-only feature);
                # avoids reading x from HBM twice
                nc.gpsimd.dma_start(x_bf[:, csl], x_sb[:, csl])
            return x_sb, x_bf

        def prep(b, x_sb, x_bf):
            st = {}
            # qkv_full: projection output staging; q lives at partitions
            # 0:16 (= packing strip 0), kpre at 32:48, vpre at 64:128.
            # Pooled k+v land in kv_sb at the SAME partition rows, so both
            # pool in a single pair of DVE ops per chunk (cost is free-size
            # bound, the extra partitions ride free).
            qkv_full = qkv.tile([C, HW], BF16, tag="qkvfull", name=f"qf_{b}")
            q_rep = qkv.tile([128, HW], BF16, tag="qrep", name=f"qr_{b}")
            kv_sb = qkv.tile([128, M], BF16, tag="k", name=f"kv_{b}")

            for cc in range(NCHUNKS):
                sl = slice(cc * NCH, (cc + 1) * NCH)
                ps_p = (ps_m if cc % 2 == 0 else ps_w).tile(
                    [128, NCH], FP32, tag="wm",
                    name=f"pj_{b}_{cc}",
                )
                nc.tensor.matmul(
                    ps_p[:], wcatT[:], x_bf[:, sl], start=True, stop=True
                )
                # one PSUM->SBUF drain (only one PSUM read operand is legal
                # per DVE op, so pooling must run from SBUF)
                nc.vector.tensor_copy(qkv_full[:, sl], ps_p[:])
                # maxpool 2x2: h-pairs first (contiguous last dim), then
                # w-pairs; engine APs at base!=0 cannot span >their 32/64
                # block, so k (rows 32:48) and v (rows 64:128) pool separately
                kv1 = qkv.tile([128, 4 * 64], BF16, tag="kv1", name=f"kv1_{b}_{cc}")
                for lo, hi in ((32, 32 + CQ), (64, 128)):
                    pp = qkv_full[lo:hi, sl].rearrange(
                        "p (h2 two w) -> p h2 two w", h2=4, two=2, w=64
                    )
                    s1 = kv1[lo:hi, :].rearrange("p (h w) -> p h w", h=4, w=64)
                    nc.vector.tensor_tensor(
                        s1, pp[:, :, 0, :], pp[:, :, 1, :], AluOpType.max
                    )
                    s1w = kv1[lo:hi, :].rearrange(
                        "p (h w2 two) -> p h w2 two", h=4, w2=32, two=2
                    )
                    s2 = kv_sb[lo:hi, cc * 128 : (cc + 1) * 128].rearrange(
                        "p (h w2) -> p h w2", h=4, w2=32
                    )
                    nc.vector.tensor_tensor(
                        s2, s1w[:, :, :, 0], s1w[:, :, :, 1], AluOpType.max
                    )

            # replicate q (per half-of-chunk-range) and k (per chunk) to
            # the other packing strips; pooled k already sits at kv_sb rows
            # 32:48 (= strip 1); other strips go into k_rep (rows 64:128 of
            # kv_sb hold v). Chunk granularity so scores start early.
            k_rep = qkv.tile([128, M], BF16, tag="krep", name=f"kr_{b}")
            if PACK:
                for h in range(2):
                    kh = slice(h * M // 2, (h + 1) * M // 2)
                    for s in (0, 2, 3):
                        nc.sync.dma_start(
                            k_rep[32 * s : 32 * s + CQ, kh],
                            kv_sb[32 : 32 + CQ, kh],
                        )
                    hsl = slice(h * EXPSPAN, (h + 1) * EXPSPAN)
                    for s in range(1, 4):
                        nc.sync.dma_start(
                            q_rep[32 * s : 32 * s + CQ, hsl],
                            qkv_full[0:CQ, hsl],
                        )
            else:
                nc.sync.dma_start(
                    k_rep[0:CQ, :], kv_sb[32 : 32 + CQ, :]
                )

            st.update(x_sb=x_sb, qkv_full=qkv_full, q_rep=q_rep, k_sb=kv_sb,
                      k_rep=k_rep, vT=[None] * MT, b=b)
            return st

        def emit_vt(st, j):
            # vT~ strip j: [128, 65] bf16, col 64 = ones (v lives at rows
            # 64:128 of kv_sb; identity block rows 64:128 matches its base).
            # Emitted lazily (interleaved into the first scores half) so the
            # in-order PE stream does not gate scores behind transposes.
            b, kv_sb = st["b"], st["k_sb"]
            ps_t = ps_w.tile([128, NCH * 2], BF16, tag="wm", name=f"tp_{b}_{j}")
            nc.tensor.transpose(
                ps_t[:, 0:CV],
                kv_sb[64:128, j * 128 : (j + 1) * 128],
                ident_bf[64:128, 64:128],
            )
            vt = vtpool.tile([128, CV + 1], BF16, tag="vt", name=f"vt_{b}_{j}")
            nc.vector.tensor_copy(vt[:, 0:CV], ps_t[:, 0:CV])
            nc.vector.memset(vt[:, CV : CV + 1], 1.0)
            st["vT"][j] = vt

        def scores_half(b, h, st, interleave):
            qkv_full, q_rep, k_sb = st["qkv_full"], st["q_rep"], st["k_sb"]
            k_rep = st["k_rep"]
            # scores + exp: 4-way row-group packed matmuls; two 2-bank
            # score buffers per m-tile so exp double-buffers against PE
            p_tiles = [
                ppool.tile([128, EXPSPAN], BF16, tag="p", name=f"p_{b}_{t}_{h}")
                for t in range(MT)
            ]
            for t in range(MT):
                interleave(t)
                sA = ps_s.tile([128, 2 * NCH], FP32, tag="s", name=f"sA_{b}_{h}_{t}")
                sB = ps_s.tile([128, 2 * NCH], FP32, tag="s", name=f"sB_{b}_{h}_{t}")
                tiles = [sA, sA, sB, sB]
                if PACK:
                    if True:
                        for i in range(4):
                            ncol = h * EXPSPAN + i * NCH
                            qsrc = qkv_full if i == 0 else q_rep
                            ksrc = k_sb if i == 1 else k_rep
                            nc.tensor.matmul(
                                tiles[i][:, (i % 2) * NCH : (i % 2 + 1) * NCH],
                                ksrc[
                                    32 * i : 32 * i + CQ,
                                    t * 128 : (t + 1) * 128,
                                ],
                                qsrc[32 * i : 32 * i + CQ, ncol : ncol + NCH],
                                start=True,
                                stop=True,
                                tile_position=(32 * i, 0),
                            )
                else:
                    for i in range(4):
                        ncol = h * EXPSPAN + i * NCH
                        nc.tensor.matmul(
                            tiles[i][:, (i % 2) * NCH : (i % 2 + 1) * NCH],
                            k_rep[0:CQ, t * 128 : (t + 1) * 128],
                            qkv_full[0:CQ, ncol : ncol + NCH],
                            start=True,
                            stop=True,
                        )
                nc.scalar.activation(
                    p_tiles[t][:, 0 : 2 * NCH], sA[:], AFT.Exp
                )
                nc.scalar.activation(
                    p_tiles[t][:, 2 * NCH : 4 * NCH], sB[:], AFT.Exp
                )

            return p_tiles

        def av_piece(b, h, st, p_tiles, ep, c, piece):
            vT = st["vT"]
            if piece == 0:
                ep["avps"][c] = ps_av.tile(
                    [128, NCH], FP32, tag="av", name=f"av_{b}_{h}_{c}"
                )
            o_ps = ep["avps"][c]
            for t in range(piece * MT // 2, (piece + 1) * MT // 2):
                nc.tensor.matmul(
                    o_ps[0 : CV + 1, :],
                    vT[t][:],
                    p_tiles[t][:, c * NCH : (c + 1) * NCH],
                    start=(t == 0),
                    stop=(t == MT - 1),
                )
            if piece == 1:
                # single drain: rows 0:64 = unnormalized AV, row 64 = den
                ou = eppool.tile([CV + 1, NCH], BF16, tag="ou", bufs=10,
                                 name=f"ou_{b}_{h}_{c}")
                nc.vector.tensor_copy(ou[:], o_ps[0 : CV + 1, :])
                dtile, drow = ep["dmap"](c)
                nc.sync.dma_start(dtile[drow : drow + 1, :], ou[CV : CV + 1, :])
                ep["ou"].append(ou)

        def av_chunk(b, h, st, p_tiles, ep, c):
            av_piece(b, h, st, p_tiles, ep, c, 0)
            av_piece(b, h, st, p_tiles, ep, c, 1)

        def den_chain(b, h, ep, c0=0, c1=4):
            # one packed reciprocal for the half's denominators [c0:c1]
            # (the dstage tile for this range always starts at partition 0)
            n = c1 - c0
            dtile, _ = ep["dmap"](c0)
            rdn = eppool.tile([4, NCH], FP32, tag="rdn", bufs=2,
                              name=f"rdn_{b}_{h}_{c0}")
            nc.vector.reciprocal(rdn[0:n, :], dtile[0:n, :])
            if "rd4" not in ep:
                ep["rd4"] = dscratch.tile([4, NCH], FP32, tag="rd",
                                          name=f"rd4_{b}_{h}")
            nc.sync.dma_start(ep["rd4"][c0:c1, :], rdn[0:n, :])

        def epilogue_chunk(b, h, st, ep, c):
            x_sb = st["x_sb"]
            cc = h * 4 + c
            sl = slice(cc * NCH, (cc + 1) * NCH)
            if "den" not in ep:
                ep["den"] = eppool.tile([CV, 4 * NCH], FP32, tag="den", bufs=2,
                                        name=f"den_{b}_{h}")
                ep["denq"] = set()
            q = c // 2
            if q not in ep["denq"]:
                ep["denq"].add(q)
                den = ep["den"]
                nc.sync.dma_start(
                    den[:, q * 2 * NCH : (q + 1) * 2 * NCH].rearrange(
                        "p (c n) -> p c n", c=2, n=NCH
                    ),
                    ep["rd4"][2 * q : 2 * q + 2, :].partition_broadcast(CV),
                )
            onorm = eppool.tile([CV, NCH], BF16, tag="onorm", bufs=3,
                                name=f"on_{b}_{h}_{c}")
            nc.vector.scalar_tensor_tensor(
                onorm[:],
                ep["ou"][c][0:CV, :],
                g_sb[0:CV, 0:1],
                ep["den"][:, c * NCH : (c + 1) * NCH],
                AluOpType.mult,
                AluOpType.mult,
            )
            o2_ps = ps_w.tile([128, NCH], FP32, tag="wm", name=f"o2_{b}_{h}_{c}")
            nc.tensor.matmul(o2_ps[:], woT[:], onorm[:], start=True, stop=True)
            out_sb = outpool.tile([C, NCH], FP32, tag="out",
                                  name=f"os_{b}_{h}_{c}")
            nc.vector.tensor_tensor(
                out_sb[:], o2_ps[:], x_sb[:, sl], AluOpType.add
            )
            nc.gpsimd.dma_start(out_e[b, :, sl], out_sb[:])

        # software-pipelined emission: prep(b+1) is emitted between the two
        # halves of batch b so its DMA/DVE/PE work fills batch b's exp phase
        # software-pipelined emission: the previous half's AV chunks are
        # emitted between the current half's score m-tiles so the PE stream
        # has work while exp drains each score buffer; prep(1) is emitted
        # after the first half so its work fills batch 0's exp phase.
        x0 = load_x(0)
        x1 = load_x(1)
        states = {0: prep(0, *x0)}
        seq = [(0, 0), (0, 1), (1, 0), (1, 1)]
        prev = None

        def make_interleave(cur_st, cur_h, prev, prev2):
            def hook(t):
                if cur_h == 0 and cur_st["vT"][t] is None:
                    emit_vt(cur_st, t)
                if prev is not None:
                    pb, ph, pst, ppt, pep = prev
                    if t in (1, 3, 5, 7):
                        av_chunk(pb, ph, pst, ppt, pep, (t - 1) // 2)
                    if t == 7:
                        den_chain(pb, ph, pep)
                if prev2 is not None and t in (0, 2, 4, 6):
                    # epilogue of the half before last: its rden round-trip
                    # is long since complete, so these never stall PE
                    epilogue_chunk(prev2[0], prev2[1], prev2[2], prev2[4], t // 2)
            return hook

        def make_ep(b, h, split):
            if split:
                tiles = [
                    eppool.tile([2, NCH], BF16, tag="dstage", bufs=3,
                                name=f"dst_{b}_{h}_{q}")
                    for q in range(2)
                ]
                return {"dmap": lambda c: (tiles[c // 2], c % 2), "ou": [], "avps": {}}
            tile = eppool.tile([4, NCH], BF16, tag="dstage", bufs=3,
                               name=f"dst_{b}_{h}")
            return {"dmap": lambda c: (tile, c), "ou": [], "avps": {}}

        prev2 = None
        for idx, (b, h) in enumerate(seq):
            st = states[b]
            ep = make_ep(b, h, (b, h) == (1, 1))
            p_tiles = scores_half(b, h, st, make_interleave(st, h, prev, prev2))
            if idx == 0:
                states[1] = prep(1, *x1)
            prev2 = prev
            prev = (b, h, st, p_tiles, ep)

        # drain the two in-flight halves; the final half goes quarter by
        # quarter so its epilogue chains start as early as possible
        for c in range(4):
            epilogue_chunk(prev2[0], prev2[1], prev2[2], prev2[4], c)
        for c in range(2):
            av_chunk(prev[0], prev[1], prev[2], prev[3], prev[4], c)
        den_chain(prev[0], prev[1], prev[4], 0, 2)
        for c in range(2):
            av_chunk(prev[0], prev[1], prev[2], prev[3], prev[4], 2 + c)
        for c in range(2):
            epilogue_chunk(prev[0], prev[1], prev[2], prev[4], c)
        den_chain(prev[0], prev[1], prev[4], 2, 4)
        for c in range(2):
            epilogue_chunk(prev[0], prev[1], prev[2], prev[4], 2 + c)

    nc.finalize()
    return nc


_NC_CACHE = None


def _get_nc():
    global _NC_CACHE
    if _NC_CACHE is None:
        _NC_CACHE = build_nc()
    return _NC_CACHE


def kernel(**inputs) -> np.ndarray:
    from concourse.bass_utils import run_bass_kernel_spmd

    x = np.asarray(inputs["x"], dtype=np.float32).reshape(B_FULL, C, HW)
    wq = np.asarray(inputs["wq"], dtype=np.float32)
    wk = np.asarray(inputs["wk"], dtype=np.float32)
    wv = np.asarray(inputs["wv"], dtype=np.float32)
    wo = np.asarray(inputs["wo"], dtype=np.float32)
    gamma = np.asarray(inputs["gamma"], dtype=np.float32)

    nc = _get_nc()
    in_maps = []
    for i in range(N_CORES):
        in_maps.append(
            {
                "x": np.ascontiguousarray(x[i * B_LOC : (i + 1) * B_LOC]),
                "wq": wq,
                "wk": wk,
                "wv": wv,
                "wo": wo,
                "gamma": gamma,
            }
        )
    res = run_bass_kernel_spmd(nc, in_maps, core_ids=list(range(N_CORES)))
    outs = [res.results[i]["out"].reshape(B_LOC, C, H, W) for i in range(N_CORES)]
    return np.concatenate(outs, axis=0)


if __name__ == "__main__":
    import reference

    inputs = {k: np.asarray(v) for k, v in reference.setup_inputs().items()}
    expected = np.asarray(reference.reference(**inputs))
    actual = kernel(**inputs)
    err = np.linalg.norm(actual - expected) / np.linalg.norm(expected)
    print("Relative error:", err)
